# revision 6
# baseline (speedup 1.0000x reference)
"""CPGCN (2-layer GCN + two FC heads) on 8 Trainium2 NeuronCores.

Destination nodes sharded across 8 cores (degree-sorted, 49 tiles of 128
per core).  Per layer: each core computes its shard of the scaled hidden
table (x@W1 * dinv), all-gathers it (node-major), builds a feature-major
SBUF-resident copy (PE pair-transposes), bulk-gathers messages with the
POOL-engine ap_gather (two int16 index streams, one per partition half),
then scatters per 128-message chunk with weighted one-hot mask matmuls
(mask = (iota==col)*w via one tensor_scalar) accumulating in PSUM.
Host does integer/layout work and final row unpermutation only.
"""
import sys
import numpy as np

sys.path.insert(0, "/opt/trn_rl_repo")

P = 128
N_NODES = 50000
N_CORES = 8
NODES_PER_CORE = N_NODES // N_CORES              # 6250
T = (NODES_PER_CORE + P - 1) // P                # 49 tiles
NP_PAD = T * P                                   # 6272
V_TAB = N_CORES * NP_PAD                         # 50176
VH = V_TAB // 2                                  # 25088 nodes per table half
NT_HALF = VH // P                                # 196 node-tiles per half
NFEAT, NHID = 512, 64
NCLS = 48
KB = NFEAT // P                                  # 4 contraction chunks


def _preprocess(edge_index, edge_weight):
    """Integer/layout-only prep: permutation, destination-major w slots (for
    degree), and message-major 2-half gather streams for ap_gather."""
    row = np.asarray(edge_index[0], dtype=np.int64)
    col = np.asarray(edge_index[1], dtype=np.int64)
    w = np.asarray(edge_weight, dtype=np.float32)
    loops = np.arange(N_NODES, dtype=np.int64)
    row = np.concatenate([row, loops])
    col = np.concatenate([col, loops])
    w = np.concatenate([w, np.ones(N_NODES, np.float32)])

    indeg = np.bincount(col, minlength=N_NODES)
    perm = np.full((N_CORES, NP_PAD), -1, np.int64)
    pos = np.empty(N_NODES, np.int64)
    for c in range(N_CORES):
        nodes = np.arange(c * NODES_PER_CORE, (c + 1) * NODES_PER_CORE)
        order = nodes[np.argsort(-indeg[nodes], kind="stable")]
        perm[c, :NODES_PER_CORE] = order
        pos[order] = c * NP_PAD + np.arange(NODES_PER_CORE)

    src_pos = pos[row]
    dst_pos = pos[col]
    dst_core = dst_pos // NP_PAD
    dst_local = dst_pos % NP_PAD
    dst_tile = dst_local // P
    dst_part = dst_local % P
    half = (src_pos >= VH).astype(np.int64)

    # ---- destination-major w slots (degree only) ----
    counts = np.zeros((N_CORES, T, P), np.int64)
    np.add.at(counts, (dst_core, dst_tile, dst_part), 1)
    K = counts.max(axis=(0, 2))
    CT = int(K.sum())
    col_off = np.concatenate([[0], np.cumsum(K)])[:-1]
    w_slots = np.zeros((N_CORES, P, CT), np.float32)
    order = np.lexsort((dst_part, dst_tile, dst_core))
    oc, ot, op_, ow = dst_core[order], dst_tile[order], dst_part[order], w[order]
    grp = (oc * T + ot) * P + op_
    gstart = np.flatnonzero(np.r_[True, grp[1:] != grp[:-1]])
    glen = np.diff(np.r_[gstart, len(grp)])
    krank = np.arange(len(grp)) - np.repeat(gstart, glen)
    w_slots[oc, op_, col_off[ot] + krank] = ow
    for c in range(N_CORES):                      # dummy nodes: deg = 1
        dummy = np.flatnonzero(perm[c] < 0)
        if len(dummy):
            dt_, dp_ = dummy // P, dummy % P
            cur = w_slots[c, dp_, col_off[dt_]]
            w_slots[c, dp_, col_off[dt_]] = np.where(cur == 0, 1.0, cur)

    # ---- message-major 2-half streams, chunked in 128s per tile ----
    # per (core, tile, half) message counts -> shared chunk schedule C[t]
    hcounts = np.zeros((N_CORES, T, 2), np.int64)
    np.add.at(hcounts, (dst_core, dst_tile, half), 1)
    C = np.maximum(-(-hcounts[..., 0].max(axis=0) // P),
                   -(-hcounts[..., 1].max(axis=0) // P))   # [T]
    CC = int(C.sum())
    c_off = np.concatenate([[0], np.cumsum(C)])[:-1]

    # stream arrays: per half: idx (elem within half), per column: col(dest
    # partition) + w.  layout [cores, P, CC] message-major: column j holds
    # 128 messages (partition = message slot within chunk).
    idxA = np.zeros((N_CORES, P, CC), np.int16)
    idxB = np.zeros((N_CORES, P, CC), np.int16)
    colA = np.zeros((N_CORES, P, CC), np.float32)
    colB = np.zeros((N_CORES, P, CC), np.float32)
    wA = np.zeros((N_CORES, P, CC), np.float32)
    wB = np.zeros((N_CORES, P, CC), np.float32)
    # order messages by (core, tile, half) then sequence within group
    order2 = np.lexsort((half, dst_tile, dst_core))
    oc2, ot2, oh2 = dst_core[order2], dst_tile[order2], half[order2]
    osrc2, ow2, opart2 = src_pos[order2], w[order2], dst_part[order2]
    grp2 = (oc2 * T + ot2) * 2 + oh2
    gstart2 = np.flatnonzero(np.r_[True, grp2[1:] != grp2[:-1]])
    glen2 = np.diff(np.r_[gstart2, len(grp2)])
    seq = np.arange(len(grp2)) - np.repeat(gstart2, glen2)
    mcol = c_off[ot2] + seq // P                  # stream column
    mrow = seq % P                                # partition (slot in chunk)
    selA = oh2 == 0
    idxA[oc2[selA], mrow[selA], mcol[selA]] = osrc2[selA].astype(np.int16)
    colA[oc2[selA], mrow[selA], mcol[selA]] = opart2[selA]
    wA[oc2[selA], mrow[selA], mcol[selA]] = ow2[selA]
    selB = ~selA
    idxB[oc2[selB], mrow[selB], mcol[selB]] = (osrc2[selB] - VH).astype(np.int16)
    colB[oc2[selB], mrow[selB], mcol[selB]] = opart2[selB]
    wB[oc2[selB], mrow[selB], mcol[selB]] = ow2[selB]

    # int16 wrapped layout for ap_gather: stream element i of column j sits
    # at [i % 16, (j*128 + i) // 16]; groups 0-3 (partition rows 0..63) get
    # stream A, groups 4-7 get stream B.
    def wrap(idx_half):     # [cores, P, CC] -> [cores, 16, CC*8]
        s = idx_half.transpose(0, 2, 1).reshape(N_CORES, CC * P)    # stream order
        return s.reshape(N_CORES, CC * 8, 16).transpose(0, 2, 1)
    wa, wb = wrap(idxA), wrap(idxB)
    idx16 = np.concatenate([np.tile(wa, (1, 4, 1)).reshape(N_CORES, 64, CC * 8),
                            np.tile(wb, (1, 4, 1)).reshape(N_CORES, 64, CC * 8)],
                           axis=1)                # [cores, 128, CC*8]

    meta = dict(K=K, col_off=col_off, CT=CT, C=C, c_off=c_off, CC=CC,
                perm=perm, pos=pos)
    streams = dict(idx16=idx16, colA=colA, colB=colB, wA=wA, wB=wB,
                   w_slots=w_slots)
    return streams, meta


def _build_nc(meta):
    import concourse.bacc as bacc
    import concourse.tile as tile
    import concourse.mybir as mybir

    dt = mybir.dt
    op = mybir.AluOpType
    K, col_off, CT = meta["K"], meta["col_off"], meta["CT"]
    C, c_off, CC = meta["C"], meta["c_off"], meta["CC"]

    nc = bacc.Bacc(None, target_bir_lowering=False)
    xT = nc.dram_tensor("xT", [T, KB, P, P], dt.float32, kind="ExternalInput")
    W1 = nc.dram_tensor("W1", [KB, P, NHID], dt.float32, kind="ExternalInput")
    W2 = nc.dram_tensor("W2", [NHID, NHID], dt.float32, kind="ExternalInput")
    fcW = nc.dram_tensor("fcW", [NHID, NCLS], dt.float32, kind="ExternalInput")
    b1r = nc.dram_tensor("b1r", [P, NHID], dt.float32, kind="ExternalInput")
    b2r = nc.dram_tensor("b2r", [P, NHID], dt.float32, kind="ExternalInput")
    fcbr = nc.dram_tensor("fcbr", [P, NCLS], dt.float32, kind="ExternalInput")
    iden = nc.dram_tensor("iden", [P, P], dt.float32, kind="ExternalInput")
    iota = nc.dram_tensor("iota", [P, P], dt.float32, kind="ExternalInput")
    wsl = nc.dram_tensor("wsl", [P, CT], dt.float32, kind="ExternalInput")
    idx16 = nc.dram_tensor("idx16", [P, CC * 8], dt.int16, kind="ExternalInput")
    colA = nc.dram_tensor("colA", [P, CC], dt.float32, kind="ExternalInput")
    colB = nc.dram_tensor("colB", [P, CC], dt.float32, kind="ExternalInput")
    wA = nc.dram_tensor("wA", [P, CC], dt.float32, kind="ExternalInput")
    wB = nc.dram_tensor("wB", [P, CC], dt.float32, kind="ExternalInput")
    out = nc.dram_tensor("out", [NP_PAD, NCLS], dt.float32, kind="ExternalOutput")

    with tile.TileContext(nc) as tc:
        with (
            tc.tile_pool(name="const", bufs=1) as cpool,
            tc.tile_pool(name="work", bufs=3) as pool,
            tc.tile_pool(name="gath", bufs=2) as gpool,
            tc.tile_pool(name="psum", bufs=2, space="PSUM") as ppool,
            tc.tile_pool(name="ptr", bufs=2, space="PSUM") as ptpool,
            tc.tile_pool(name="dram", bufs=1, space="DRAM") as dram,
        ):
            w_res = cpool.tile([P, CT], dt.float32)
            nc.sync.dma_start(w_res[:], wsl[:])
            idx_res = cpool.tile([P, CC * 8], dt.int16)
            nc.sync.dma_start(idx_res[:], idx16[:])
            colA_res = cpool.tile([P, CC], dt.float32)
            nc.sync.dma_start(colA_res[:], colA[:])
            colB_res = cpool.tile([P, CC], dt.float32)
            nc.sync.dma_start(colB_res[:], colB[:])
            wA_res = cpool.tile([P, CC], dt.float32)
            nc.sync.dma_start(wA_res[:], wA[:])
            wB_res = cpool.tile([P, CC], dt.float32)
            nc.sync.dma_start(wB_res[:], wB[:])
            W1_sb = [cpool.tile([P, NHID], dt.float32, name=f"w1_{kb}", tag=f"w1_{kb}")
                     for kb in range(KB)]
            for kb in range(KB):
                nc.sync.dma_start(W1_sb[kb][:], W1[kb])
            W2_sb = cpool.tile([NHID, NHID], dt.float32)
            nc.sync.dma_start(W2_sb[:], W2[:])
            fcW_sb = cpool.tile([NHID, NCLS], dt.float32)
            nc.sync.dma_start(fcW_sb[:], fcW[:])
            b1_sb = cpool.tile([P, NHID], dt.float32)
            nc.sync.dma_start(b1_sb[:], b1r[:])
            b2_sb = cpool.tile([P, NHID], dt.float32)
            nc.sync.dma_start(b2_sb[:], b2r[:])
            fcb_sb = cpool.tile([P, NCLS], dt.float32)
            nc.sync.dma_start(fcb_sb[:], fcbr[:])
            iden_sb = cpool.tile([P, P], dt.float32)
            nc.sync.dma_start(iden_sb[:], iden[:])
            iota_sb = cpool.tile([P, P], dt.float32)
            nc.sync.dma_start(iota_sb[:], iota[:])
            dinv_sb = cpool.tile([P, T], dt.float32)
            tabF = cpool.tile([P, VH], dt.float32)   # feature-major table

            agin1 = dram.tile([NP_PAD, NHID], dt.float32)
            agout1 = dram.tile([V_TAB, NHID], dt.float32, addr_space="Shared")
            agin2 = dram.tile([NP_PAD, NHID], dt.float32)
            agout2 = dram.tile([V_TAB, NHID], dt.float32, addr_space="Shared")

            # ---- phase A: deg/dinv + xws shard ----
            for t in range(T):
                o, k = int(col_off[t]), int(K[t])
                deg = pool.tile([P, 1], dt.float32, tag="deg")
                nc.vector.tensor_reduce(
                    out=deg[:], in_=w_res[:, o:o + k],
                    axis=mybir.AxisListType.X, op=op.add)
                nc.scalar.activation(
                    out=deg[:], in_=deg[:],
                    func=mybir.ActivationFunctionType.Sqrt)
                nc.vector.reciprocal(out=dinv_sb[:, t:t + 1], in_=deg[:])
                pxw = ppool.tile([P, NHID], dt.float32, tag="misc")
                for kb in range(KB):
                    xt_t = pool.tile([P, P], dt.float32, tag="xt")
                    nc.sync.dma_start(xt_t[:], xT[t, kb])
                    nc.tensor.matmul(pxw[:], lhsT=xt_t[:], rhs=W1_sb[kb][:],
                                     start=(kb == 0), stop=(kb == KB - 1))
                xws = pool.tile([P, NHID], dt.float32, tag="xws")
                nc.vector.tensor_scalar(
                    out=xws[:], in0=pxw[:], scalar1=dinv_sb[:, t:t + 1],
                    scalar2=None, op0=op.mult)
                nc.sync.dma_start(agin1[t * P:(t + 1) * P, :], xws[:])

            nc.gpsimd.collective_compute(
                "AllGather", op.bypass,
                replica_groups=[list(range(N_CORES))],
                ins=[agin1[:].opt()], outs=[agout1[:].opt()])

            def build_table(agout):
                """agout [V_TAB, 64] node-major -> tabF [128, VH] feature-major
                (rows 0:64 = lo-half nodes' features, 64:128 = hi-half)."""
                for n in range(NT_HALF):
                    pair = pool.tile([P, P], dt.float32, tag="pair")
                    nc.sync.dma_start(pair[:, 0:NHID], agout[n * P:(n + 1) * P, :])
                    nc.sync.dma_start(pair[:, NHID:P],
                                      agout[VH + n * P:VH + (n + 1) * P, :])
                    ptr = ptpool.tile([P, P], dt.float32, tag="ptr")
                    nc.tensor.transpose(ptr[:], pair[:], iden_sb[:])
                    nc.scalar.activation(
                        out=tabF[:, n * P:(n + 1) * P], in_=ptr[:],
                        func=mybir.ActivationFunctionType.Copy)

            def propagate(t, bias_sb, relu):
                o, cn = int(c_off[t]), int(C[t])
                g = gpool.tile([P, cn * P], dt.float32, tag="g")
                nc.gpsimd.ap_gather(
                    out_ap=g[:].unsqueeze(2),
                    in_ap=tabF[:].unsqueeze(2),
                    idxs_ap=idx_res[:, o * 8:(o + cn) * 8],
                    channels=P, num_elems=VH, d=1, num_idxs=cn * P)
                ps = ppool.tile([P, NHID], dt.float32, tag="ps")
                for c in range(cn):
                    j = o + c
                    ptr = ptpool.tile([P, P], dt.float32, tag="ptr")
                    nc.tensor.transpose(ptr[:], g[:, c * P:(c + 1) * P], iden_sb[:])
                    gt = pool.tile([P, P], dt.float32, tag="gt")
                    nc.scalar.activation(
                        out=gt[:], in_=ptr[:],
                        func=mybir.ActivationFunctionType.Copy)
                    mA = pool.tile([P, P], dt.float32, tag="mA")
                    nc.vector.tensor_scalar(
                        out=mA[:], in0=iota_sb[:],
                        scalar1=colA_res[:, j:j + 1], scalar2=wA_res[:, j:j + 1],
                        op0=op.is_equal, op1=op.mult)
                    nc.tensor.matmul(ps[:], lhsT=mA[:], rhs=gt[:, 0:NHID],
                                     start=(c == 0), stop=False)
                    mB = pool.tile([P, P], dt.float32, tag="mB")
                    nc.vector.tensor_scalar(
                        out=mB[:], in0=iota_sb[:],
                        scalar1=colB_res[:, j:j + 1], scalar2=wB_res[:, j:j + 1],
                        op0=op.is_equal, op1=op.mult)
                    nc.tensor.matmul(ps[:], lhsT=mB[:], rhs=gt[:, NHID:P],
                                     start=False, stop=(c == cn - 1))
                red = pool.tile([P, NHID], dt.float32, tag="red")
                nc.vector.tensor_scalar(
                    out=red[:], in0=ps[:], scalar1=dinv_sb[:, t:t + 1],
                    scalar2=None, op0=op.mult)
                nc.vector.tensor_tensor(out=red[:], in0=red[:], in1=bias_sb[:], op=op.add)
                if relu:
                    nc.vector.tensor_scalar(
                        out=red[:], in0=red[:], scalar1=0.0, scalar2=None, op0=op.max)
                pT = ppool.tile([NHID, P], dt.float32, tag="misc")
                nc.tensor.transpose(pT[:], red[:], iden_sb[:])
                hT = pool.tile([NHID, P], dt.float32, tag="hT")
                nc.vector.tensor_copy(out=hT[:], in_=pT[:])
                return hT

            # ---- layer 1 ----
            build_table(agout1)
            for t in range(T):
                hT = propagate(t, b1_sb, relu=True)
                pxw2 = ppool.tile([P, NHID], dt.float32, tag="misc")
                nc.tensor.matmul(pxw2[:], lhsT=hT[:], rhs=W2_sb[:],
                                 start=True, stop=True)
                hs = pool.tile([P, NHID], dt.float32, tag="hs")
                nc.vector.tensor_scalar(
                    out=hs[:], in0=pxw2[:], scalar1=dinv_sb[:, t:t + 1],
                    scalar2=None, op0=op.mult)
                nc.sync.dma_start(agin2[t * P:(t + 1) * P, :], hs[:])

            nc.gpsimd.collective_compute(
                "AllGather", op.bypass,
                replica_groups=[list(range(N_CORES))],
                ins=[agin2[:].opt()], outs=[agout2[:].opt()])

            # ---- layer 2 ----
            build_table(agout2)
            for t in range(T):
                h2T = propagate(t, b2_sb, relu=False)
                po = ppool.tile([P, NCLS], dt.float32, tag="misc")
                nc.tensor.matmul(po[:], lhsT=h2T[:], rhs=fcW_sb[:],
                                 start=True, stop=True)
                ot_ = pool.tile([P, NCLS], dt.float32, tag="ot")
                nc.vector.tensor_tensor(out=ot_[:], in0=po[:], in1=fcb_sb[:], op=op.add)
                nc.sync.dma_start(out[t * P:(t + 1) * P, :], ot_[:])

    nc.finalize()
    return nc


def kernel(x, edge_index, edge_weight, W1, b1, W2, b2, fcW1, fcb1, fcW2, fcb2):
    from concourse.bass_utils import run_bass_kernel_spmd

    x = np.asarray(x, np.float32)
    W1 = np.asarray(W1, np.float32)
    W2 = np.asarray(W2, np.float32)
    b1 = np.asarray(b1, np.float32)
    b2 = np.asarray(b2, np.float32)
    fcW = np.concatenate([np.asarray(fcW1, np.float32), np.asarray(fcW2, np.float32)], axis=1)
    fcb = np.concatenate([np.asarray(fcb1, np.float32), np.asarray(fcb2, np.float32)])

    streams, meta = _preprocess(edge_index, edge_weight)
    nc = _build_nc(meta)

    W1_in = np.ascontiguousarray(W1.reshape(KB, P, NHID))
    b1r = np.tile(b1[None, :], (P, 1)).astype(np.float32)
    b2r = np.tile(b2[None, :], (P, 1)).astype(np.float32)
    fcbr = np.tile(fcb[None, :], (P, 1)).astype(np.float32)
    iden = np.eye(P, dtype=np.float32)
    iota = np.tile(np.arange(P, dtype=np.float32)[None, :], (P, 1))
    perm, pos = meta["perm"], meta["pos"]

    in_maps = []
    for c in range(N_CORES):
        xp = np.zeros((NP_PAD, NFEAT), np.float32)
        valid = perm[c] >= 0
        xp[valid] = x[perm[c][valid]]
        xT = np.ascontiguousarray(xp.reshape(T, P, KB, P).transpose(0, 2, 3, 1))
        in_maps.append({
            "xT": xT, "W1": W1_in, "W2": W2, "fcW": fcW,
            "b1r": b1r, "b2r": b2r, "fcbr": fcbr, "iden": iden, "iota": iota,
            "wsl": np.ascontiguousarray(streams["w_slots"][c]),
            "idx16": np.ascontiguousarray(streams["idx16"][c]),
            "colA": np.ascontiguousarray(streams["colA"][c]),
            "colB": np.ascontiguousarray(streams["colB"][c]),
            "wA": np.ascontiguousarray(streams["wA"][c]),
            "wB": np.ascontiguousarray(streams["wB"][c]),
        })

    res = run_bass_kernel_spmd(nc, in_maps, core_ids=list(range(N_CORES)))
    full = np.concatenate([res.results[c]["out"] for c in range(N_CORES)], axis=0)
    full = full[pos]
    return full[:, :16].copy(), full[:, 16:].copy()


# revision 7
# speedup vs baseline: 1.5721x; 1.5721x over previous
"""CPGCN (2-layer GCN + two FC heads) on 8 Trainium2 NeuronCores.

Sharding: destination nodes are sharded across the 8 cores (6250 each,
degree-sorted within a core and padded to 6272 = 49 tiles of 128).
Each core computes xw = x @ W1 for its own nodes, scales by dinv
(symmetric GCN norm), all-gathers the scaled hidden table, then gathers
messages for its own destinations via indirect DMA using a padded-CSR
slot layout (host-built, shared per-tile K schedule across cores),
multiplies per-slot weights, reduces, applies bias/relu, repeats for
layer 2, and applies the two FC heads on device.  Host only does integer
index/layout work and the final row unpermutation.
"""
import sys
import numpy as np

sys.path.insert(0, "/opt/trn_rl_repo")

P = 128
N_NODES = 50000
N_CORES = 8
NODES_PER_CORE = N_NODES // N_CORES              # 6250
T = (NODES_PER_CORE + P - 1) // P                # 49 tiles
NP_PAD = T * P                                   # 6272
V_TAB = N_CORES * NP_PAD                         # 50176
NFEAT, NHID = 512, 64
NCLS = 48                                        # 16 + 32 concat
KB = NFEAT // P                                  # 4 contraction chunks


def _preprocess(edge_index, edge_weight):
    """Integer/layout-only prep: permutation, padded-CSR slots (single CSR,
    int32 indices into the global device-ordered table)."""
    row = np.asarray(edge_index[0], dtype=np.int64)
    col = np.asarray(edge_index[1], dtype=np.int64)
    w = np.asarray(edge_weight, dtype=np.float32)
    loops = np.arange(N_NODES, dtype=np.int64)
    row = np.concatenate([row, loops])
    col = np.concatenate([col, loops])
    w = np.concatenate([w, np.ones(N_NODES, np.float32)])

    indeg = np.bincount(col, minlength=N_NODES)
    perm = np.full((N_CORES, NP_PAD), -1, np.int64)
    pos = np.empty(N_NODES, np.int64)
    for c in range(N_CORES):
        nodes = np.arange(c * NODES_PER_CORE, (c + 1) * NODES_PER_CORE)
        order = nodes[np.argsort(-indeg[nodes], kind="stable")]
        perm[c, :NODES_PER_CORE] = order
        pos[order] = c * NP_PAD + np.arange(NODES_PER_CORE)

    src_pos = pos[row]
    dst_pos = pos[col]
    dst_core = dst_pos // NP_PAD
    dst_local = dst_pos % NP_PAD
    dst_tile = dst_local // P
    dst_part = dst_local % P

    counts = np.zeros((N_CORES, T, P), np.int64)
    np.add.at(counts, (dst_core, dst_tile, dst_part), 1)
    K = counts.max(axis=(0, 2))                  # [T] shared K schedule
    CT = int(K.sum())
    col_off = np.concatenate([[0], np.cumsum(K)])[:-1]

    idx_slots = np.zeros((N_CORES, P, CT), np.int32)   # pad -> row 0 (w=0)
    w_slots = np.zeros((N_CORES, P, CT), np.float32)
    order = np.lexsort((dst_part, dst_tile, dst_core))
    oc, ot, op_ = dst_core[order], dst_tile[order], dst_part[order]
    osrc, ow = src_pos[order], w[order]
    grp = (oc * T + ot) * P + op_
    first = np.r_[True, grp[1:] != grp[:-1]]
    gstart = np.flatnonzero(first)
    glen = np.diff(np.r_[gstart, len(grp)])
    krank = np.arange(len(grp)) - np.repeat(gstart, glen)
    colpos = col_off[ot] + krank
    idx_slots[oc, op_, colpos] = osrc.astype(np.int32)
    w_slots[oc, op_, colpos] = ow

    # dummy nodes (perm == -1): give one unit slot so deg=1 -> dinv=1 (finite);
    # their x rows are zero so table rows stay zero.
    for c in range(N_CORES):
        dummy_local = np.flatnonzero(perm[c] < 0)
        if len(dummy_local):
            dt_, dp_ = dummy_local // P, dummy_local % P
            w_slots[c, dp_, col_off[dt_]] = np.where(
                w_slots[c, dp_, col_off[dt_]] == 0, 1.0, w_slots[c, dp_, col_off[dt_]])
    return idx_slots, w_slots, K, col_off, CT, perm, pos


def _build_nc(K, col_off, CT):
    import concourse.bacc as bacc
    import concourse.tile as tile
    import concourse.mybir as mybir
    from concourse.bass import IndirectOffsetOnAxis

    dt = mybir.dt
    op = mybir.AluOpType

    nc = bacc.Bacc(None, target_bir_lowering=False)
    xT = nc.dram_tensor("xT", [T, KB, P, P], dt.float32, kind="ExternalInput")
    W1 = nc.dram_tensor("W1", [KB, P, NHID], dt.float32, kind="ExternalInput")
    W2 = nc.dram_tensor("W2", [NHID, NHID], dt.float32, kind="ExternalInput")
    fcW = nc.dram_tensor("fcW", [NHID, NCLS], dt.float32, kind="ExternalInput")
    b1r = nc.dram_tensor("b1r", [P, NHID], dt.float32, kind="ExternalInput")
    b2r = nc.dram_tensor("b2r", [P, NHID], dt.float32, kind="ExternalInput")
    fcbr = nc.dram_tensor("fcbr", [P, NCLS], dt.float32, kind="ExternalInput")
    iden = nc.dram_tensor("iden", [P, P], dt.float32, kind="ExternalInput")
    idxs = nc.dram_tensor("idxs", [P, CT], dt.int32, kind="ExternalInput")
    wsl = nc.dram_tensor("wsl", [P, CT], dt.float32, kind="ExternalInput")
    out = nc.dram_tensor("out", [NP_PAD, NCLS], dt.float32, kind="ExternalOutput")

    with tile.TileContext(nc) as tc:
        with (
            tc.tile_pool(name="const", bufs=1) as cpool,
            tc.tile_pool(name="work", bufs=3) as pool,
            tc.tile_pool(name="gath", bufs=2) as gpool,
            tc.tile_pool(name="psum", bufs=2, space="PSUM") as ppool,
            tc.tile_pool(name="dram", bufs=1, space="DRAM") as dram,
        ):
            # resident constants
            w_res = cpool.tile([P, CT], dt.float32)
            idx_res = cpool.tile([P, CT], dt.int32)
            nc.sync.dma_start(w_res[:], wsl[:])
            nc.sync.dma_start(idx_res[:], idxs[:])
            W1_sb = [cpool.tile([P, NHID], dt.float32, name=f"w1_{kb}", tag=f"w1_{kb}")
                     for kb in range(KB)]
            for kb in range(KB):
                nc.sync.dma_start(W1_sb[kb][:], W1[kb])
            W2_sb = cpool.tile([NHID, NHID], dt.float32)
            nc.sync.dma_start(W2_sb[:], W2[:])
            fcW_sb = cpool.tile([NHID, NCLS], dt.float32)
            nc.sync.dma_start(fcW_sb[:], fcW[:])
            b1_sb = cpool.tile([P, NHID], dt.float32)
            nc.sync.dma_start(b1_sb[:], b1r[:])
            b2_sb = cpool.tile([P, NHID], dt.float32)
            nc.sync.dma_start(b2_sb[:], b2r[:])
            fcb_sb = cpool.tile([P, NCLS], dt.float32)
            nc.sync.dma_start(fcb_sb[:], fcbr[:])
            iden_sb = cpool.tile([P, P], dt.float32)
            nc.sync.dma_start(iden_sb[:], iden[:])
            dinv_sb = cpool.tile([P, T], dt.float32)

            agin1 = dram.tile([NP_PAD, NHID], dt.float32)
            agout1 = dram.tile([V_TAB, NHID], dt.float32, addr_space="Shared")
            agin2 = dram.tile([NP_PAD, NHID], dt.float32)
            agout2 = dram.tile([V_TAB, NHID], dt.float32, addr_space="Shared")

            # ---- phase A: deg/dinv + xws ----
            for t in range(T):
                o, k = int(col_off[t]), int(K[t])
                deg = pool.tile([P, 1], dt.float32, tag="deg")
                nc.vector.tensor_reduce(
                    out=deg[:], in_=w_res[:, o:o + k],
                    axis=mybir.AxisListType.X, op=op.add)
                nc.scalar.activation(
                    out=deg[:], in_=deg[:],
                    func=mybir.ActivationFunctionType.Sqrt)
                nc.vector.reciprocal(out=dinv_sb[:, t:t + 1], in_=deg[:])
                pxw = ppool.tile([P, NHID], dt.float32, tag="pxw")
                for kb in range(KB):
                    xt_t = pool.tile([P, P], dt.float32, tag="xt")
                    nc.sync.dma_start(xt_t[:], xT[t, kb])
                    nc.tensor.matmul(pxw[:], lhsT=xt_t[:], rhs=W1_sb[kb][:],
                                     start=(kb == 0), stop=(kb == KB - 1))
                xws = pool.tile([P, NHID], dt.float32, tag="xws")
                nc.vector.tensor_scalar(
                    out=xws[:], in0=pxw[:], scalar1=dinv_sb[:, t:t + 1],
                    scalar2=None, op0=op.mult)
                nc.sync.dma_start(agin1[t * P:(t + 1) * P, :], xws[:])

            # ---- phase B: all-gather layer-1 table ----
            nc.gpsimd.collective_compute(
                "AllGather", op.bypass,
                replica_groups=[list(range(N_CORES))],
                ins=[agin1[:].opt()], outs=[agout1[:].opt()])

            def propagate(agout, t, bias_sb, relu):
                o, k = int(col_off[t]), int(K[t])
                g = gpool.tile([P, k * NHID], dt.float32, tag="g")
                for j in range(k):
                    nc.gpsimd.indirect_dma_start(
                        out=g[:, j * NHID:(j + 1) * NHID], out_offset=None,
                        in_=agout[:],
                        in_offset=IndirectOffsetOnAxis(
                            ap=idx_res[:, o + j:o + j + 1], axis=0))
                g3 = g[:].rearrange("p (k d) -> p k d", d=NHID)
                wb = w_res[:, o:o + k].unsqueeze(2).to_broadcast([P, k, NHID])
                nc.vector.tensor_tensor(out=g3, in0=g3, in1=wb, op=op.mult)
                red = pool.tile([P, NHID], dt.float32, tag="red")
                nc.vector.tensor_reduce(
                    out=red[:], in_=g[:].rearrange("p (k d) -> p d k", d=NHID),
                    axis=mybir.AxisListType.X, op=op.add)
                # h = (red * dinv + bias) [relu]
                nc.vector.tensor_scalar(
                    out=red[:], in0=red[:], scalar1=dinv_sb[:, t:t + 1],
                    scalar2=None, op0=op.mult)
                nc.vector.tensor_tensor(out=red[:], in0=red[:], in1=bias_sb[:], op=op.add)
                if relu:
                    nc.vector.tensor_scalar(
                        out=red[:], in0=red[:], scalar1=0.0, scalar2=None, op0=op.max)
                # transpose -> [64, 128]
                pT = ppool.tile([NHID, P], dt.float32, tag="pT")
                nc.tensor.transpose(pT[:], red[:], iden_sb[:])
                hT = pool.tile([NHID, P], dt.float32, tag="hT")
                nc.vector.tensor_copy(out=hT[:], in_=pT[:])
                return hT

            # ---- phase C: layer-1 propagate + W2 -> hs ----
            for t in range(T):
                hT = propagate(agout1, t, b1_sb, relu=True)
                pxw2 = ppool.tile([P, NHID], dt.float32, tag="pxw2")
                nc.tensor.matmul(pxw2[:], lhsT=hT[:], rhs=W2_sb[:],
                                 start=True, stop=True)
                hs = pool.tile([P, NHID], dt.float32, tag="hs")
                nc.vector.tensor_scalar(
                    out=hs[:], in0=pxw2[:], scalar1=dinv_sb[:, t:t + 1],
                    scalar2=None, op0=op.mult)
                nc.sync.dma_start(agin2[t * P:(t + 1) * P, :], hs[:])

            # ---- phase D: all-gather layer-2 table ----
            nc.gpsimd.collective_compute(
                "AllGather", op.bypass,
                replica_groups=[list(range(N_CORES))],
                ins=[agin2[:].opt()], outs=[agout2[:].opt()])

            # ---- phase E: layer-2 propagate + FC heads ----
            for t in range(T):
                h2T = propagate(agout2, t, b2_sb, relu=False)
                po = ppool.tile([P, NCLS], dt.float32, tag="po")
                nc.tensor.matmul(po[:], lhsT=h2T[:], rhs=fcW_sb[:],
                                 start=True, stop=True)
                ot_ = pool.tile([P, NCLS], dt.float32, tag="ot")
                nc.vector.tensor_tensor(out=ot_[:], in0=po[:], in1=fcb_sb[:], op=op.add)
                nc.sync.dma_start(out[t * P:(t + 1) * P, :], ot_[:])

    nc.finalize()
    return nc


def kernel(x, edge_index, edge_weight, W1, b1, W2, b2, fcW1, fcb1, fcW2, fcb2):
    from concourse.bass_utils import run_bass_kernel_spmd

    x = np.asarray(x, np.float32)
    W1 = np.asarray(W1, np.float32)
    W2 = np.asarray(W2, np.float32)
    b1 = np.asarray(b1, np.float32)
    b2 = np.asarray(b2, np.float32)
    fcW = np.concatenate([np.asarray(fcW1, np.float32), np.asarray(fcW2, np.float32)], axis=1)
    fcb = np.concatenate([np.asarray(fcb1, np.float32), np.asarray(fcb2, np.float32)])

    idx_slots, w_slots, K, col_off, CT, perm, pos = _preprocess(edge_index, edge_weight)

    nc = _build_nc(K, col_off, CT)

    W1_in = np.ascontiguousarray(W1.reshape(KB, P, NHID))
    b1r = np.tile(b1[None, :], (P, 1)).astype(np.float32)
    b2r = np.tile(b2[None, :], (P, 1)).astype(np.float32)
    fcbr = np.tile(fcb[None, :], (P, 1)).astype(np.float32)
    iden = np.eye(P, dtype=np.float32)

    in_maps = []
    for c in range(N_CORES):
        xp = np.zeros((NP_PAD, NFEAT), np.float32)
        valid = perm[c] >= 0
        xp[valid] = x[perm[c][valid]]
        # xT[t, kb] = xp[t*128:(t+1)*128, kb*128:(kb+1)*128].T
        xT = np.ascontiguousarray(
            xp.reshape(T, P, KB, P).transpose(0, 2, 3, 1))
        in_maps.append({
            "xT": xT, "W1": W1_in, "W2": W2, "fcW": fcW,
            "b1r": b1r, "b2r": b2r, "fcbr": fcbr, "iden": iden,
            "idxs": np.ascontiguousarray(idx_slots[c]),
            "wsl": np.ascontiguousarray(w_slots[c]),
        })

    res = run_bass_kernel_spmd(nc, in_maps, core_ids=list(range(N_CORES)))
    full = np.concatenate([res.results[c]["out"] for c in range(N_CORES)], axis=0)
    full = full[pos]                              # unpermute to original node order
    return full[:, :16].copy(), full[:, 16:].copy()
